# revision 18
# baseline (speedup 1.0000x reference)
"""Trainium2 Bass kernel for the KGEncoder RGCN (nn_KGEncoder_14027363188782).

Math (per batch element b, L=5 layers):
    x0 = ent_emb                                             (E, D)
    per layer i:
      y_r   = x @ Wb_x[i,r] + 1 * c[i,r]^T    (E, NB)  where c[i,r] = rel_r @ Wb_rel[i,r]
      Z     = sum_r adj_r @ y_r               (E, NB)
      h     = relu(Z @ Ww[i] + bias[i])
      g     = sigmoid(h @ Wh[i] + bh[i])
      x     = x + g * (h - x)
    out_b = sum_e x[e] * m[e] / max(sum_e m[e], 1)

Sharding: core c handles b = c // 2 (pair-replicated, no collectives).

Big matmul orientation: stationary = adjT 256x128 DoubleRow blocks
(fp8, exact for 0/1), moving = y chunk (256, 3) -> psum (128 i, 3).
Z chunks are PE-transposed back to (3, E) for the basis/highway tail.
"""

import numpy as np
import ml_dtypes

import concourse.bacc as bacc
import concourse.bass as bass
import concourse.mybir as mybir
import concourse.tile as tile
from concourse import bass_utils
from concourse.bass import MemorySpace

B, R, E, D, HID, L, NB = 4, 10, 1500, 100, 100, 5, 3
EP = 1536           # entity (j) dim padded to 12*128
CH = EP // 128      # 12 j-chunks of 128
C2 = 6              # 256-row contraction chunks (DoubleRow)
E2 = 1504           # i dim padded (16-aligned, 11.75 chunks -> last is 96)
IC = 12             # i chunks of 128 (last covers 1504-1408=96... see ISLICE)
YQ = 32             # y_all per-chunk col stride (16-aligned)
RNB = R * NB        # 30
f32 = mybir.dt.float32
bf16 = mybir.dt.bfloat16
fp8 = mybir.dt.float8e4
AF = mybir.ActivationFunctionType
AX = mybir.AxisListType
DR = mybir.MatmulPerfMode.DoubleRow
f32r = mybir.dt.float32r

# i chunking: 12 chunks; chunks 0..10 are 128 wide, chunk 11 is 96 (1504 total)
ISL = [(k * 128, min(128, E2 - k * 128)) for k in range(IC)]
# free-dim chunking of E2 for the tail (psum bank = 512 f32)
NSL = [(0, 512), (512, 512), (1024, 480)]

# f32 const blob layout (cols), partitions used in parens
OF_IDF = 0            # identity f32 (128)
OF_WBX = 128          # (100) L*RNB
OF_WBR = 278          # (100)
OF_BIAS = 428         # (100) L
OF_BH = 433           # (100) L
OF_RELT = 438         # (100) R
OF_MASK = 448         # (128) CH
OF_ONES = 460         # (1) 128 ones row
OF_ONES128 = 588      # (128) 1 ones column
OF_XT0 = 589          # (100) EP initial x (ent_emb.T padded)
CBF = 589 + EP
# bf16 const blob layout
OF_IDH = 0            # identity bf16 (128)
OF_WW = 128           # (3) L*HID
OF_WH = 628           # (100) L*HID
CBH = 1128

_NC_CACHE = {}


def _build_nc():
    nc = bacc.Bacc("TRN2", target_bir_lowering=False, debug=False)

    adjT = nc.dram_tensor("adjT", [R, C2, 128, 2, E2], fp8, kind="ExternalInput").ap()
    cstFD = nc.dram_tensor("cstF", [128, CBF], f32, kind="ExternalInput").ap()
    cstHD = nc.dram_tensor("cstH", [128, CBH], bf16, kind="ExternalInput").ap()
    graphD = nc.dram_tensor("graph", [HID, 1], f32, kind="ExternalOutput").ap()

    with tile.TileContext(nc) as tc:
        with (
            tc.tile_pool(name="singles", bufs=1) as singles,
            tc.tile_pool(name="resp", bufs=1) as resp,
            tc.tile_pool(name="ypool", bufs=2) as ypool,
            tc.tile_pool(name="workp", bufs=2) as workp,
            tc.tile_pool(name="psY", bufs=1, space=MemorySpace.PSUM) as psY,
            tc.tile_pool(name="psZ", bufs=1, space=MemorySpace.PSUM) as psZ,
            tc.tile_pool(name="psT", bufs=1, space=MemorySpace.PSUM) as psT,
            tc.tile_pool(name="psB", bufs=2, space=MemorySpace.PSUM) as psB,
        ):
            # ---- small persistent state (loaded before the big adj DMAs) ----
            cstF = singles.tile([128, CBF], f32, tag="cstF", name="cstF")
            nc.sync.dma_start(out=cstF[:, :], in_=cstFD)
            cstH = singles.tile([128, CBH], bf16, tag="cstH", name="cstH")
            nc.sync.dma_start(out=cstH[:, :], in_=cstHD)
            xT = singles.tile([D, EP], f32, tag="xT", name="xT")
            nc.gpsimd.tensor_copy(xT[:, :], cstF[0:D, OF_XT0 : OF_XT0 + EP])

            wbx_sb = cstF[0:D, OF_WBX : OF_WBX + L * RNB]
            wbr_sb = cstF[0:D, OF_WBR : OF_WBR + L * RNB]
            bias_sb = cstF[0:HID, OF_BIAS : OF_BIAS + L]
            bh_sb = cstF[0:HID, OF_BH : OF_BH + L]
            relT_sb = cstF[0:D, OF_RELT : OF_RELT + R]
            mask_sb = cstF[0:128, OF_MASK : OF_MASK + CH]
            ones = cstF[0:1, OF_ONES : OF_ONES + 128]
            ones128 = cstF[0:128, OF_ONES128 : OF_ONES128 + 1]
            identF = cstF[0:128, OF_IDF : OF_IDF + 128]
            ident_sb = cstH[0:128, OF_IDH : OF_IDH + 128]
            ww_sb = cstH[0:NB, OF_WW : OF_WW + L * HID]
            wh_sb = cstH[0:HID, OF_WH : OF_WH + L * HID]

            # masked-mean denominator: den = max(sum(mask), 1); rden = 1/den
            mrow = workp.tile([128, 1], f32, tag="mrow", name="mrow", bufs=1)
            nc.vector.reduce_sum(mrow[:, :], mask_sb, axis=AX.X)
            den_ps = psB.tile([HID, 512], f32, tag="hh", bufs=1, name="den_ps")
            nc.tensor.matmul(
                den_ps[0:1, 0:1], ones128, mrow[:, :], start=True, stop=True,
            )
            den1 = workp.tile([1, 1], f32, tag="den", name="den", bufs=1)
            nc.vector.tensor_scalar_max(den1[:, :], den_ps[0:1, 0:1], 1.0)
            nc.vector.reciprocal(den1[:, :], den1[:, :])
            psd = psB.tile([HID, 512], f32, tag="hh", bufs=1, name="psd")
            nc.tensor.matmul(
                psd[:, 0:1], cstF[0:1, OF_ONES : OF_ONES + HID], den1[:, :],
                start=True, stop=True,
            )
            rden = singles.tile([HID, 1], f32, tag="rden", name="rden")
            nc.scalar.copy(out=rden[:, :], in_=psd[:, 0:1])

            # ---- resident adjT relations: (p, (c t i)) fp8 ----
            res_tiles = []
            for r in range(R):
                rt = resp.tile([128, C2 * 2 * E2], fp8, tag=f"res{r}", name=f"res{r}")
                nc.sync.dma_start(
                    out=rt[:, :].rearrange("p (c t i) -> p c t i", c=C2, t=2),
                    in_=adjT[r].rearrange("c p t i -> p c t i"),
                )
                res_tiles.append(rt)
            res_views = [
                res_tiles[r][:, :].rearrange("p (c t i) -> p c t i", c=C2, t=2)
                for r in range(R)
            ]

            # ---- layers ----
            for i in range(L):
                wbx_i = cstF[0:D, OF_WBX + i * RNB : OF_WBX + (i + 1) * RNB]
                wbr_i = cstF[0:D, OF_WBR + i * RNB : OF_WBR + (i + 1) * RNB]
                ww_i = cstH[0:NB, OF_WW + i * HID : OF_WW + (i + 1) * HID]
                wh_i = cstH[0:HID, OF_WH + i * HID : OF_WH + (i + 1) * HID]
                bias_i = cstF[0:HID, OF_BIAS + i : OF_BIAS + i + 1]
                bh_i = cstF[0:HID, OF_BH + i : OF_BH + i + 1]

                # c[r, :] = rel_r @ Wb_rel[i, r] -> psy partition 0, tail cols
                psy = psY.tile([128, CH * RNB + YQ], f32, tag="y", name=f"psy{i}")
                for r in range(R):
                    nc.tensor.matmul(
                        psy[0:1, CH * RNB + 3 * r : CH * RNB + 3 * r + 3],
                        cstF[0:D, OF_RELT + r : OF_RELT + r + 1],
                        wbr_i[:, 3 * r : 3 * r + 3],
                        start=True, stop=True,
                    )
                c_sb = workp.tile([1, RNB], f32, tag="c_sb", name=f"c_sb{i}", bufs=2)
                nc.scalar.copy(out=c_sb[:, :], in_=psy[0:1, CH * RNB : CH * RNB + RNB])

                # y[kchunk] = x[kchunk] @ Wbx[i] + 1 (x) c  -> fp8 (128, 30)/chunk
                for k in range(CH):
                    ks = slice(k * RNB, (k + 1) * RNB)
                    nc.tensor.matmul(
                        psy[:, ks], xT[:, k * 128 : (k + 1) * 128], wbx_i,
                        start=True, stop=False,
                    )
                    nc.tensor.matmul(
                        psy[:, ks], cstF[0:1, OF_ONES : OF_ONES + 128], c_sb[:, :],
                        start=False, stop=True,
                    )
                y_all = ypool.tile([128, CH * YQ], fp8, tag="y_all", name=f"y_all{i}")
                nc.scalar.copy(
                    out=y_all[:, :].rearrange("p (k q) -> p k q", k=CH)[:, :, 0:RNB],
                    in_=psy[:, 0 : CH * RNB].rearrange("p (k q) -> p k q", k=CH),
                )
                y_view = y_all[:, :].rearrange("p (k q) -> p k q", q=YQ)

                # Z[ic] (128 i, 3) += adjT_block.T @ y_chunk   (DoubleRow fp8)
                # layer 0: two passes (relations 0-4 while 5-9 still loading)
                passes = [(0, 5), (5, R)] if i == 0 else [(0, R)]
                zp_tiles = []
                for pi, (r0, r1) in enumerate(passes):
                    zps = psZ.tile([128, IC * 8], f32, tag=f"z{pi}",
                                   name=f"zps{i}_{pi}")
                    for ic in range(IC):
                        i0, iw = ISL[ic]
                        for r in range(r0, r1):
                            for c in range(C2):
                                nc.tensor.matmul(
                                    zps[0:iw, ic * 8 : ic * 8 + NB],
                                    res_views[r][:, c, :, i0 : i0 + iw],
                                    y_view[:, 2 * c : 2 * c + 2, 3 * r : 3 * r + 3],
                                    start=(r == r0 and c == 0),
                                    stop=(r == r1 - 1 and c == C2 - 1),
                                    perf_mode=DR,
                                )
                    zp_tiles.append(zps)
                zc_sb = workp.tile([128, IC * NB], bf16, tag="zc",
                                   name=f"zc{i}", bufs=2)
                if i == 0:
                    zcA = workp.tile([128, IC * NB], f32, tag="zcA",
                                     name="zcA", bufs=1)
                    nc.scalar.copy(
                        out=zcA[:, :].rearrange("p (k w) -> p k w", w=NB),
                        in_=zp_tiles[0][:, :].rearrange(
                            "p (k w) -> p k w", w=8)[:, :, 0:NB])
                    nc.vector.tensor_add(
                        zc_sb[:, :].rearrange("p (k w) -> p k w", w=NB),
                        zcA[:, :].rearrange("p (k w) -> p k w", w=NB),
                        zp_tiles[1][:, :].rearrange(
                            "p (k w) -> p k w", w=8)[:, :, 0:NB])
                else:
                    nc.scalar.copy(
                        out=zc_sb[:, :].rearrange("p (k w) -> p k w", w=NB),
                        in_=zp_tiles[0][:, :].rearrange(
                            "p (k w) -> p k w", w=8)[:, :, 0:NB])

                # transpose Z chunks -> zT (3, E2) bf16
                zt01 = psT.tile([NB, 1024], bf16, tag="zt0", name=f"zt0_{i}",
                                bufs=1)
                zt2 = psT.tile([NB, 512], bf16, tag="zt2", name=f"zt2_{i}",
                               bufs=1)
                for ic in range(IC):
                    i0, iw = ISL[ic]
                    pst = zt01 if ic < 8 else zt2
                    off = (ic % 8) * 128
                    nc.tensor.transpose(
                        pst[:, off : off + iw],
                        zc_sb[0:iw, ic * NB : (ic + 1) * NB],
                        ident_sb[0:iw, 0:iw],
                    )
                zT_chunks = []
                for n in range(3):
                    n0, nw = NSL[n]
                    ztc = workp.tile([NB, 512], bf16, tag="zT", name=f"zT{i}_{n}",
                                     bufs=2)
                    src = zt01[:, n * 512 : n * 512 + nw] if n < 2 else \
                        zt2[:, 0:nw]
                    nc.vector.tensor_copy(ztc[:, 0:nw], src)
                    zT_chunks.append(ztc)

                # tail: h = relu(Z @ Ww + bias); g = sigmoid(h @ Wh + bh);
                # x += g * (h - x)
                if i == L - 1:
                    gsum_ps = psB.tile([HID, 512], f32, tag="gg", bufs=2,
                                       name="gsum_ps")
                for n in range(3):
                    n0, nw = NSL[n]
                    ns = slice(n0, n0 + nw)
                    psh = psB.tile([HID, 512], f32, tag="hh", bufs=1,
                                   name=f"psh{i}_{n0}")
                    nc.tensor.matmul(
                        psh[:, 0:nw], ww_i, zT_chunks[n][:, 0:nw],
                        start=True, stop=True,
                    )
                    hc = workp.tile([HID, 512], bf16, tag="h", name=f"h{i}_{n}",
                                    bufs=2)
                    nc.scalar.activation(
                        hc[:, 0:nw], psh[:, 0:nw], AF.Relu, bias=bias_i,
                    )
                    psg = psB.tile([HID, 512], f32, tag="gg", bufs=2,
                                   name=f"psg{i}_{n0}")
                    nc.tensor.matmul(
                        psg[:, 0:nw], wh_i, hc[:, 0:nw],
                        start=True, stop=True,
                    )
                    gc = workp.tile([HID, 512], bf16, tag="g", name=f"g{i}_{n}",
                                    bufs=2)
                    nc.scalar.activation(
                        gc[:, 0:nw], psg[:, 0:nw], AF.Sigmoid, bias=bh_i,
                    )
                    nc.vector.tensor_sub(hc[:, 0:nw], hc[:, 0:nw], xT[:, ns])
                    nc.vector.tensor_mul(hc[:, 0:nw], hc[:, 0:nw], gc[:, 0:nw])
                    nc.vector.tensor_add(xT[:, ns], xT[:, ns], hc[:, 0:nw])
                    if i == L - 1:
                        # fold masked-mean accumulation in as x chunks settle
                        for k in range(4 * n, 4 * n + 4):
                            xt_ps = psY.tile([128, CH * RNB + YQ], f32,
                                             tag="y", name=f"xtp{k}")
                            nc.tensor.transpose(
                                xt_ps[:, 0:HID],
                                xT[:, k * 128 : (k + 1) * 128],
                                cstF[0:HID, OF_IDF : OF_IDF + HID],
                            )
                            x_im = workp.tile([128, HID], f32, tag="x_im",
                                              name=f"x_im{k}", bufs=2)
                            nc.scalar.copy(out=x_im[:, :], in_=xt_ps[:, 0:HID])
                            nc.tensor.matmul(
                                gsum_ps[:, 0:1], x_im[:, :],
                                cstF[0:128, OF_MASK + k : OF_MASK + k + 1],
                                start=(k == 0), stop=(k == CH - 1),
                            )

            # ---- finish masked mean ----
            gsum = workp.tile([HID, 1], f32, tag="gsum", name="gsum", bufs=1)
            nc.vector.tensor_mul(gsum[:, :], gsum_ps[:, 0:1], rden[:, :])
            nc.sync.dma_start(out=graphD, in_=gsum[:, :])

    nc.compile()
    return nc


def get_nc():
    if "nc" not in _NC_CACHE:
        _NC_CACHE["nc"] = _build_nc()
    return _NC_CACHE["nc"]


def make_in_maps(adj, mask_ids, ent_emb, rel_emb, Wb, Ww, bias, Wh, bh):
    adj = np.asarray(adj, dtype=np.float32)
    pad = np.zeros((B, R, EP, E2), dtype=ml_dtypes.float8_e4m3fn)
    pad[:, :, :E, :E] = adj.transpose(0, 1, 3, 2).astype(ml_dtypes.float8_e4m3fn)
    # [b, r, c, p, t, i] = adj[b, r, i, j = c*256 + t*128 + p]
    adjT = np.ascontiguousarray(
        pad.reshape(B, R, C2, 2, 128, E2).transpose(0, 1, 2, 4, 3, 5)
    )
    entT = np.zeros((D, EP), dtype=np.float32)
    entT[:, :E] = np.asarray(ent_emb, np.float32).T
    relTh = np.ascontiguousarray(np.asarray(rel_emb, np.float32).T)
    Wb5 = np.asarray(Wb, np.float32).reshape(L, R, 2, D, NB)
    wbx = np.ascontiguousarray(
        Wb5[:, :, 0].transpose(0, 2, 1, 3).reshape(L, D, RNB)
    )
    wbr = np.ascontiguousarray(
        Wb5[:, :, 1].transpose(0, 2, 1, 3).reshape(L, D, RNB)
    )
    maskf = np.asarray(mask_ids).astype(np.float32)
    cstF_ = np.zeros((128, CBF), np.float32)
    cstF_[0:128, OF_IDF:OF_IDF+128] = np.eye(128, dtype=np.float32)
    cstF_[0:D, OF_WBX:OF_WBX+L*RNB] = wbx.transpose(1, 0, 2).reshape(D, L*RNB)
    cstF_[0:D, OF_WBR:OF_WBR+L*RNB] = wbr.transpose(1, 0, 2).reshape(D, L*RNB)
    cstF_[0:HID, OF_BIAS:OF_BIAS+L] = np.asarray(bias, np.float32).T
    cstF_[0:HID, OF_BH:OF_BH+L] = np.asarray(bh, np.float32).T
    cstF_[0:D, OF_RELT:OF_RELT+R] = relTh
    cstF_[0:1, OF_ONES:OF_ONES+128] = 1.0
    cstF_[0:128, OF_ONES128] = 1.0
    cstH_ = np.zeros((128, CBH), ml_dtypes.bfloat16)
    cstH_[0:128, OF_IDH:OF_IDH+128] = np.eye(128, dtype=np.float32)
    wwf = np.asarray(Ww, np.float32)   # (L, NB, HID)
    cstH_[0:NB, OF_WW:OF_WW+L*HID] = wwf.transpose(1, 0, 2).reshape(NB, L*HID)
    whf = np.asarray(Wh, np.float32)   # (L, HID, HID)
    cstH_[0:HID, OF_WH:OF_WH+L*HID] = whf.transpose(1, 0, 2).reshape(HID, L*HID)
    cstF_[0:D, OF_XT0:OF_XT0+EP] = entT
    common = dict(cstF=cstF_, cstH=np.ascontiguousarray(cstH_))
    in_maps = []
    for c in range(8):
        b = c // 2
        m = dict(common)
        m["adjT"] = np.ascontiguousarray(adjT[b])
        mp = np.zeros((EP,), dtype=np.float32)
        mp[:E] = maskf[b]
        cf = common["cstF"].copy()
        cf[0:128, OF_MASK:OF_MASK+CH] = mp.reshape(CH, 128).T
        m["cstF"] = cf
        in_maps.append(m)
    return in_maps


def run(inputs, trace=False):
    nc = get_nc()
    in_maps = make_in_maps(**{k: np.asarray(v) for k, v in inputs.items()})
    res = bass_utils.run_bass_kernel_spmd(
        nc, in_maps, core_ids=list(range(8)), trace=trace
    )
    out = np.stack(
        [np.asarray(res.results[2 * b]["graph"]).reshape(HID) for b in range(B)]
    ).astype(np.float32)
    return out, res


def kernel(**inputs):
    out, _ = run(inputs, trace=False)
    return out


# revision 20
# speedup vs baseline: 1.0112x; 1.0112x over previous
"""Trainium2 Bass kernel for the KGEncoder RGCN (nn_KGEncoder_14027363188782).

Math (per batch element b, L=5 layers):
    x0 = ent_emb                                             (E, D)
    per layer i:
      y_r   = x @ Wb_x[i,r] + 1 * c[i,r]^T    (E, NB)  where c[i,r] = rel_r @ Wb_rel[i,r]
      Z     = sum_r adj_r @ y_r               (E, NB)
      h     = relu(Z @ Ww[i] + bias[i])
      g     = sigmoid(h @ Wh[i] + bh[i])
      x     = x + g * (h - x)
    out_b = sum_e x[e] * m[e] / max(sum_e m[e], 1)

Sharding: core c handles b = c // 2 (pair-replicated, no collectives).

Big matmul orientation: stationary = adjT 256x128 DoubleRow blocks
(fp8, exact for 0/1), moving = y chunk (256, 3) -> psum (128 i, 3).
Z chunks are PE-transposed back to (3, E) for the basis/highway tail.
"""

import numpy as np
import ml_dtypes

import concourse.bacc as bacc
import concourse.bass as bass
import concourse.mybir as mybir
import concourse.tile as tile
from concourse import bass_utils
from concourse.bass import MemorySpace

B, R, E, D, HID, L, NB = 4, 10, 1500, 100, 100, 5, 3
EP = 1536           # entity (j) dim padded to 12*128
CH = EP // 128      # 12 j-chunks of 128
C2 = 6              # 256-row contraction chunks (DoubleRow)
E2 = 1504           # i dim padded (16-aligned, 11.75 chunks -> last is 96)
IC = 12             # i chunks of 128 (last covers 1504-1408=96... see ISLICE)
YQ = 32             # y_all per-chunk col stride (16-aligned)
RNB = R * NB        # 30
f32 = mybir.dt.float32
bf16 = mybir.dt.bfloat16
fp8 = mybir.dt.float8e4
AF = mybir.ActivationFunctionType
AX = mybir.AxisListType
DR = mybir.MatmulPerfMode.DoubleRow
f32r = mybir.dt.float32r

# i chunking: 12 chunks; chunks 0..10 are 128 wide, chunk 11 is 96 (1504 total)
ISL = [(k * 128, min(128, E2 - k * 128)) for k in range(IC)]
# free-dim chunking of E2 for the tail (psum bank = 512 f32)
NSL = [(0, 512), (512, 512), (1024, 480)]

# f32 const blob layout (cols), partitions used in parens
OF_IDF = 0            # identity f32 (128)
OF_WBX = 128          # (100) L*RNB
OF_WBR = 278          # (100)
OF_BIAS = 428         # (100) L
OF_BH = 433           # (100) L
OF_RELT = 438         # (100) R
OF_MASK = 448         # (128) CH
OF_ONES = 460         # (1) 128 ones row
OF_ONES128 = 588      # (128) 1 ones column
OF_XT0 = 589          # (100) EP initial x (ent_emb.T padded)
CBF = 589 + EP
# bf16 const blob layout
OF_IDH = 0            # identity bf16 (128)
OF_WW = 128           # (3) L*HID
OF_WH = 628           # (100) L*HID
CBH = 1128

_NC_CACHE = {}


def _build_nc():
    nc = bacc.Bacc("TRN2", target_bir_lowering=False, debug=False)

    adjT = nc.dram_tensor("adjT", [R, C2, 128, 2, E2], fp8, kind="ExternalInput").ap()
    cstFD = nc.dram_tensor("cstF", [128, CBF], f32, kind="ExternalInput").ap()
    cstHD = nc.dram_tensor("cstH", [128, CBH], bf16, kind="ExternalInput").ap()
    graphD = nc.dram_tensor("graph", [HID, 1], f32, kind="ExternalOutput").ap()

    with tile.TileContext(nc) as tc:
        with (
            tc.tile_pool(name="singles", bufs=1) as singles,
            tc.tile_pool(name="resp", bufs=1) as resp,
            tc.tile_pool(name="ypool", bufs=2) as ypool,
            tc.tile_pool(name="workp", bufs=2) as workp,
            tc.tile_pool(name="psY", bufs=1, space=MemorySpace.PSUM) as psY,
            tc.tile_pool(name="psZ", bufs=1, space=MemorySpace.PSUM) as psZ,
            tc.tile_pool(name="psT", bufs=1, space=MemorySpace.PSUM) as psT,
            tc.tile_pool(name="psB", bufs=2, space=MemorySpace.PSUM) as psB,
        ):
            # ---- small persistent state (loaded before the big adj DMAs) ----
            cstF = singles.tile([128, CBF], f32, tag="cstF", name="cstF")
            nc.sync.dma_start(out=cstF[:, :], in_=cstFD)
            cstH = singles.tile([128, CBH], bf16, tag="cstH", name="cstH")
            nc.sync.dma_start(out=cstH[:, :], in_=cstHD)
            xT = singles.tile([D, EP], f32, tag="xT", name="xT")
            nc.gpsimd.tensor_copy(xT[:, :], cstF[0:D, OF_XT0 : OF_XT0 + EP])

            wbx_sb = cstF[0:D, OF_WBX : OF_WBX + L * RNB]
            wbr_sb = cstF[0:D, OF_WBR : OF_WBR + L * RNB]
            bias_sb = cstF[0:HID, OF_BIAS : OF_BIAS + L]
            bh_sb = cstF[0:HID, OF_BH : OF_BH + L]
            relT_sb = cstF[0:D, OF_RELT : OF_RELT + R]
            mask_sb = cstF[0:128, OF_MASK : OF_MASK + CH]
            ones = cstF[0:1, OF_ONES : OF_ONES + 128]
            ones128 = cstF[0:128, OF_ONES128 : OF_ONES128 + 1]
            identF = cstF[0:128, OF_IDF : OF_IDF + 128]
            ident_sb = cstH[0:128, OF_IDH : OF_IDH + 128]
            ww_sb = cstH[0:NB, OF_WW : OF_WW + L * HID]
            wh_sb = cstH[0:HID, OF_WH : OF_WH + L * HID]

            # masked-mean denominator: den = max(sum(mask), 1); rden = 1/den
            mrow = workp.tile([128, 1], f32, tag="mrow", name="mrow", bufs=1)
            nc.vector.reduce_sum(mrow[:, :], mask_sb, axis=AX.X)
            den_ps = psB.tile([HID, 512], f32, tag="hh", bufs=1, name="den_ps")
            nc.tensor.matmul(
                den_ps[0:1, 0:1], ones128, mrow[:, :], start=True, stop=True,
            )
            den1 = workp.tile([1, 1], f32, tag="den", name="den", bufs=1)
            nc.vector.tensor_scalar_max(den1[:, :], den_ps[0:1, 0:1], 1.0)
            nc.vector.reciprocal(den1[:, :], den1[:, :])
            psd = psB.tile([HID, 512], f32, tag="hh", bufs=1, name="psd")
            nc.tensor.matmul(
                psd[:, 0:1], cstF[0:1, OF_ONES : OF_ONES + HID], den1[:, :],
                start=True, stop=True,
            )
            rden = singles.tile([HID, 1], f32, tag="rden", name="rden")
            nc.scalar.copy(out=rden[:, :], in_=psd[:, 0:1])

            # c[i, r, :] = rel_r @ Wb_rel[i, r] for all layers, hoisted
            psc = psY.tile([128, CH * RNB + YQ], f32, tag="y", name="psc_all")
            for i in range(L):
                for r in range(R):
                    nc.tensor.matmul(
                        psc[0:1, i * RNB + 3 * r : i * RNB + 3 * r + 3],
                        cstF[0:D, OF_RELT + r : OF_RELT + r + 1],
                        cstF[0:D, OF_WBR + i * RNB + 3 * r :
                             OF_WBR + i * RNB + 3 * r + 3],
                        start=True, stop=True,
                    )
            c_all = singles.tile([1, L * RNB], f32, tag="c_all", name="c_all")
            nc.scalar.copy(out=c_all[:, :], in_=psc[0:1, 0 : L * RNB])

            # ---- resident adjT relations: (p, (c t i)) fp8 ----
            res_tiles = []
            for r in range(R):
                rt = resp.tile([128, C2 * 2 * E2], fp8, tag=f"res{r}", name=f"res{r}")
                nc.sync.dma_start(
                    out=rt[:, :].rearrange("p (c t i) -> p c t i", c=C2, t=2),
                    in_=adjT[r].rearrange("c p t i -> p c t i"),
                )
                res_tiles.append(rt)
            res_views = [
                res_tiles[r][:, :].rearrange("p (c t i) -> p c t i", c=C2, t=2)
                for r in range(R)
            ]

            # ---- layers ----
            for i in range(L):
                wbx_i = cstF[0:D, OF_WBX + i * RNB : OF_WBX + (i + 1) * RNB]
                wbr_i = cstF[0:D, OF_WBR + i * RNB : OF_WBR + (i + 1) * RNB]
                ww_i = cstH[0:NB, OF_WW + i * HID : OF_WW + (i + 1) * HID]
                wh_i = cstH[0:HID, OF_WH + i * HID : OF_WH + (i + 1) * HID]
                bias_i = cstF[0:HID, OF_BIAS + i : OF_BIAS + i + 1]
                bh_i = cstF[0:HID, OF_BH + i : OF_BH + i + 1]

                psy = psY.tile([128, CH * RNB + YQ], f32, tag="y", name=f"psy{i}")
                c_sb = c_all[0:1, i * RNB : (i + 1) * RNB]

                # y[kchunk] = x[kchunk] @ Wbx[i] + 1 (x) c  -> fp8 (128, 30)/chunk
                for k in range(CH):
                    ks = slice(k * RNB, (k + 1) * RNB)
                    nc.tensor.matmul(
                        psy[:, ks], xT[:, k * 128 : (k + 1) * 128], wbx_i,
                        start=True, stop=False,
                    )
                    nc.tensor.matmul(
                        psy[:, ks], cstF[0:1, OF_ONES : OF_ONES + 128], c_sb,
                        start=False, stop=True,
                    )
                y_all = ypool.tile([128, CH * YQ], fp8, tag="y_all", name=f"y_all{i}")
                nc.scalar.copy(
                    out=y_all[:, :].rearrange("p (k q) -> p k q", k=CH)[:, :, 0:RNB],
                    in_=psy[:, 0 : CH * RNB].rearrange("p (k q) -> p k q", k=CH),
                )
                y_view = y_all[:, :].rearrange("p (k q) -> p k q", q=YQ)

                # Z[ic] (128 i, 3) += adjT_block.T @ y_chunk   (DoubleRow fp8)
                # layer 0: two passes (relations 0-4 while 5-9 still loading)
                passes = [(0, 5), (5, R)] if i == 0 else [(0, R)]
                zp_tiles = []
                for pi, (r0, r1) in enumerate(passes):
                    zps = psZ.tile([128, IC * 8], f32, tag=f"z{pi}",
                                   name=f"zps{i}_{pi}")
                    for ic in range(IC):
                        i0, iw = ISL[ic]
                        for r in range(r0, r1):
                            for c in range(C2):
                                nc.tensor.matmul(
                                    zps[0:iw, ic * 8 : ic * 8 + NB],
                                    res_views[r][:, c, :, i0 : i0 + iw],
                                    y_view[:, 2 * c : 2 * c + 2, 3 * r : 3 * r + 3],
                                    start=(r == r0 and c == 0),
                                    stop=(r == r1 - 1 and c == C2 - 1),
                                    perf_mode=DR,
                                )
                    zp_tiles.append(zps)
                zc_sb = workp.tile([128, IC * NB], bf16, tag="zc",
                                   name=f"zc{i}", bufs=2)
                if i == 0:
                    zcA = workp.tile([128, IC * NB], f32, tag="zcA",
                                     name="zcA", bufs=1)
                    nc.scalar.copy(
                        out=zcA[:, :].rearrange("p (k w) -> p k w", w=NB),
                        in_=zp_tiles[0][:, :].rearrange(
                            "p (k w) -> p k w", w=8)[:, :, 0:NB])
                    nc.vector.tensor_add(
                        zc_sb[:, :].rearrange("p (k w) -> p k w", w=NB),
                        zcA[:, :].rearrange("p (k w) -> p k w", w=NB),
                        zp_tiles[1][:, :].rearrange(
                            "p (k w) -> p k w", w=8)[:, :, 0:NB])
                else:
                    nc.scalar.copy(
                        out=zc_sb[:, :].rearrange("p (k w) -> p k w", w=NB),
                        in_=zp_tiles[0][:, :].rearrange(
                            "p (k w) -> p k w", w=8)[:, :, 0:NB])

                # transpose Z chunks -> zT (3, E2) bf16
                # tail: h = relu(Z @ Ww + bias); g = sigmoid(h @ Wh + bh);
                # x += g * (h - x); transposes pipelined per 4-ic chunk
                if i == L - 1:
                    gsum_ps = psZ.tile([128, IC * 8], f32, tag="z1", bufs=1,
                                       name="gsum_ps")
                for n in range(3):
                    n0, nw = NSL[n]
                    ns = slice(n0, n0 + nw)
                    ztp = psT.tile([NB, 512], bf16, tag=f"zt{n}",
                                   name=f"zt{n}_{i}", bufs=1)
                    for ic in range(4 * n, 4 * n + 4):
                        i0, iw = ISL[ic]
                        off = (ic % 4) * 128
                        nc.tensor.transpose(
                            ztp[:, off : off + iw],
                            zc_sb[0:iw, ic * NB : (ic + 1) * NB],
                            ident_sb[0:iw, 0:iw],
                        )
                    ztc = workp.tile([NB, 512], bf16, tag="zT", name=f"zT{i}_{n}",
                                     bufs=2)
                    nc.vector.tensor_copy(ztc[:, 0:nw], ztp[:, 0:nw])
                    psh = psB.tile([HID, 512], f32, tag="hh", bufs=1,
                                   name=f"psh{i}_{n0}")
                    nc.tensor.matmul(
                        psh[:, 0:nw], ww_i, ztc[:, 0:nw],
                        start=True, stop=True,
                    )
                    hc = workp.tile([HID, 512], bf16, tag="h", name=f"h{i}_{n}",
                                    bufs=2)
                    nc.scalar.activation(
                        hc[:, 0:nw], psh[:, 0:nw], AF.Relu, bias=bias_i,
                    )
                    psg = psB.tile([HID, 512], f32, tag="gg", bufs=1,
                                   name=f"psg{i}_{n0}")
                    nc.tensor.matmul(
                        psg[:, 0:nw], wh_i, hc[:, 0:nw],
                        start=True, stop=True,
                    )
                    gc = workp.tile([HID, 512], bf16, tag="g", name=f"g{i}_{n}",
                                    bufs=2)
                    nc.scalar.activation(
                        gc[:, 0:nw], psg[:, 0:nw], AF.Sigmoid, bias=bh_i,
                    )
                    nc.vector.tensor_sub(hc[:, 0:nw], hc[:, 0:nw], xT[:, ns])
                    nc.vector.tensor_mul(hc[:, 0:nw], hc[:, 0:nw], gc[:, 0:nw])
                    nc.vector.tensor_add(xT[:, ns], xT[:, ns], hc[:, 0:nw])
                    if i == L - 1:
                        # fold masked-mean accumulation in as x chunks settle
                        for k in range(4 * n, 4 * n + 4):
                            xt_ps = psY.tile([128, CH * RNB + YQ], f32,
                                             tag="y", name=f"xtp{k}")
                            nc.tensor.transpose(
                                xt_ps[:, 0:HID],
                                xT[:, k * 128 : (k + 1) * 128],
                                cstF[0:HID, OF_IDF : OF_IDF + HID],
                            )
                            x_im = workp.tile([128, HID], f32, tag="x_im",
                                              name=f"x_im{k}", bufs=2)
                            nc.scalar.copy(out=x_im[:, :], in_=xt_ps[:, 0:HID])
                            nc.tensor.matmul(
                                gsum_ps[0:HID, 0:1], x_im[:, :],
                                cstF[0:128, OF_MASK + k : OF_MASK + k + 1],
                                start=(k == 0), stop=(k == CH - 1),
                            )

            # ---- finish masked mean ----
            gsum = workp.tile([HID, 1], f32, tag="gsum", name="gsum", bufs=1)
            nc.vector.tensor_mul(gsum[:, :], gsum_ps[0:HID, 0:1], rden[:, :])
            nc.sync.dma_start(out=graphD, in_=gsum[:, :])

    nc.compile()
    return nc


def get_nc():
    if "nc" not in _NC_CACHE:
        _NC_CACHE["nc"] = _build_nc()
    return _NC_CACHE["nc"]


def make_in_maps(adj, mask_ids, ent_emb, rel_emb, Wb, Ww, bias, Wh, bh):
    adj = np.asarray(adj, dtype=np.float32)
    pad = np.zeros((B, R, EP, E2), dtype=ml_dtypes.float8_e4m3fn)
    pad[:, :, :E, :E] = adj.transpose(0, 1, 3, 2).astype(ml_dtypes.float8_e4m3fn)
    # [b, r, c, p, t, i] = adj[b, r, i, j = c*256 + t*128 + p]
    adjT = np.ascontiguousarray(
        pad.reshape(B, R, C2, 2, 128, E2).transpose(0, 1, 2, 4, 3, 5)
    )
    entT = np.zeros((D, EP), dtype=np.float32)
    entT[:, :E] = np.asarray(ent_emb, np.float32).T
    relTh = np.ascontiguousarray(np.asarray(rel_emb, np.float32).T)
    Wb5 = np.asarray(Wb, np.float32).reshape(L, R, 2, D, NB)
    wbx = np.ascontiguousarray(
        Wb5[:, :, 0].transpose(0, 2, 1, 3).reshape(L, D, RNB)
    )
    wbr = np.ascontiguousarray(
        Wb5[:, :, 1].transpose(0, 2, 1, 3).reshape(L, D, RNB)
    )
    maskf = np.asarray(mask_ids).astype(np.float32)
    cstF_ = np.zeros((128, CBF), np.float32)
    cstF_[0:128, OF_IDF:OF_IDF+128] = np.eye(128, dtype=np.float32)
    cstF_[0:D, OF_WBX:OF_WBX+L*RNB] = wbx.transpose(1, 0, 2).reshape(D, L*RNB)
    cstF_[0:D, OF_WBR:OF_WBR+L*RNB] = wbr.transpose(1, 0, 2).reshape(D, L*RNB)
    cstF_[0:HID, OF_BIAS:OF_BIAS+L] = np.asarray(bias, np.float32).T
    cstF_[0:HID, OF_BH:OF_BH+L] = np.asarray(bh, np.float32).T
    cstF_[0:D, OF_RELT:OF_RELT+R] = relTh
    cstF_[0:1, OF_ONES:OF_ONES+128] = 1.0
    cstF_[0:128, OF_ONES128] = 1.0
    cstH_ = np.zeros((128, CBH), ml_dtypes.bfloat16)
    cstH_[0:128, OF_IDH:OF_IDH+128] = np.eye(128, dtype=np.float32)
    wwf = np.asarray(Ww, np.float32)   # (L, NB, HID)
    cstH_[0:NB, OF_WW:OF_WW+L*HID] = wwf.transpose(1, 0, 2).reshape(NB, L*HID)
    whf = np.asarray(Wh, np.float32)   # (L, HID, HID)
    cstH_[0:HID, OF_WH:OF_WH+L*HID] = whf.transpose(1, 0, 2).reshape(HID, L*HID)
    cstF_[0:D, OF_XT0:OF_XT0+EP] = entT
    common = dict(cstF=cstF_, cstH=np.ascontiguousarray(cstH_))
    in_maps = []
    for c in range(8):
        b = c // 2
        m = dict(common)
        m["adjT"] = np.ascontiguousarray(adjT[b])
        mp = np.zeros((EP,), dtype=np.float32)
        mp[:E] = maskf[b]
        cf = common["cstF"].copy()
        cf[0:128, OF_MASK:OF_MASK+CH] = mp.reshape(CH, 128).T
        m["cstF"] = cf
        in_maps.append(m)
    return in_maps


def run(inputs, trace=False):
    nc = get_nc()
    in_maps = make_in_maps(**{k: np.asarray(v) for k, v in inputs.items()})
    res = bass_utils.run_bass_kernel_spmd(
        nc, in_maps, core_ids=list(range(8)), trace=trace
    )
    out = np.stack(
        [np.asarray(res.results[2 * b]["graph"]).reshape(HID) for b in range(B)]
    ).astype(np.float32)
    return out, res


def kernel(**inputs):
    out, _ = run(inputs, trace=False)
    return out
